# revision 34
# baseline (speedup 1.0000x reference)
"""Complex Conv1D (VALID, stride 1) on Trainium2 — Bass/Tile, 8-core data-parallel.

Problem (hardcoded shapes):
  x_real/x_imag: [32, 4096, 64] f32, kernel_real/imag: [9, 64, 64] f32,
  bias_real/imag: [64] f32  ->  out [32, 4088, 64, 2] f32
  out_real = conv(xr, wr) - conv(xi, wi) + br
  out_imag = conv(xr, wi) + conv(xi, wr) + bi

Mapping: complex multiply as its 2x2 real block-matrix form so each tap is ONE
full 128-contract matmul:
  X_b [128, L]   rows 0:64 = xr[b].T (channels on partitions), 64:128 = xi[b].T
  W[k] [128,128] = [[wr[k], wi[k]], [-wi[k], wr[k]]]
  psum[128, T] += W[k].T @ X_b[:, l0+k : l0+k+T]   for k = 0..8
  psum rows 0:64 = real output (filters), rows 64:128 = imag output.
Batch is sharded 4-per-core across 8 cores; weights replicated. The kernel
emits the output transposed as [b, 128, L_out]; the host restores
[B, L_out, F, 2].

PE does 9 rows (128x128 MACs each) per output position — 9*4088*4 = 147k rows
per core = 61.4us at 2.4GHz, the hard floor. The rest of the design keeps the
PE near that floor (HW-measured choices, each A/B'd via a repeat-loop diff):
  - bf16 operands (same 1 cycle/row as f32r, half the SBUF/DMA traffic;
    rel err ~2.3e-3 vs the f32 reference, gate is 2e-2). f32 outputs.
  - whole-batch X tiles DMA'd in 8 chunks on the SP queue: spreading the
    transfers reduced measured DMA<->PE SBUF contention vs one big burst.
  - evacuation psum->SBUF on the DVE (vector) engine, not Act: measured
    ~5us less PE interference; out-DMAs ride the Act queue; bias load on
    the gpsimd/SWDGE path to keep the startup HWDGE queue clear.
  - first X chunk small (512 cols) so the first matmul group starts ~3us
    in; warmup matmuls measured net-negative (they delay the real stream
    more than the p-state ramp costs), so warmup defaults to 0.
  - 6 PSUM banks cycling; 3 X buffers / 4 out buffers for prefetch depth.
"""

import numpy as np

import concourse.bacc as bacc
import concourse.bass as bass
import concourse.mybir as mybir
from concourse.ap import AP
from concourse.tile import TileContext
from concourse.bass_utils import run_bass_kernel_spmd

B, L, CIN, KT, F = 32, 4096, 64, 9, 64
LOUT = L - KT + 1  # 4088
NCORES = 8
BPC = B // NCORES  # batches per core
TL = 512  # output-tile width (one PSUM bank of fp32)
NLT = (LOUT + TL - 1) // TL  # 8

MM_DT_NAME = "bfloat16"
OUT_DT_NAME = "float32"

# Partial-fp8 scheme (fp8taps=2): taps {0,1} run as one fp8-e4m3 DoubleRow
# matmul pair (contraction 256, 2x PE rate), taps 2..8 stay bf16. Scales keep
# one PSUM accumulation consistent: fp8 X is pre-scaled by 2^XSH and fp8 W by
# 2^WSH (product 2^PSH), bf16-tap weights and the bias by 2^PSH, and the host
# multiplies the output by 2^-PSH. Measured rel err 1.78e-2 (gate 2e-2).
XSH, WSH = 4, 6
PSH = XSH + WSH

# Default build config. bf16 taps only: the partial-fp8 DoubleRow path
# (fp8taps=2) is correct on HW (rel err 1.78e-2) and ~5.6us faster in the
# cost-model sim, but measured +3us SLOWER on hardware — DoubleRow does not
# deliver 2x per-column throughput on this silicon/runtime, so the extra xq
# DMAs and the two-bank evacuation are pure overhead. Kept as an option.
# The micro-opts here are sim-positive and HW-noise-neutral: thinned final
# tile (shorter end-of-kernel DMA drain), split W prologue DMA + first X
# chunk on the idle ACT queue (parallel prologue), all 8 PSUM banks.
DEFAULT_BUILD_KW = {
    "fp8taps": 0,
    "tail": 256,
    "wsplit": 1,
    "x0eng": "scalar",
    "psbufs": 8,
}


def _build_nc(
    mm_dt,
    w_dt=None,
    out_dt=None,
    xbufs=3,
    obufs=4,
    psbufs=6,
    warmup=0,
    warmup_rows=512,
    xchunks=8,
    first_chunk=512,
    xeng="sync",
    out_eng="scalar",
    bias_eng="gpsimd",
    tail=0,
    korder=0,
    evac="dve",
    repeat=1,
    loop_repeat=None,
    wsplit=0,
    x0eng=None,
    fp8taps=0,
):
    nc = bacc.Bacc("TRN2", target_bir_lowering=False, debug=False, num_devices=NCORES)
    if w_dt is None:
        w_dt = mm_dt
    if out_dt is None:
        out_dt = getattr(mybir.dt, OUT_DT_NAME)

    x_d = nc.dram_tensor("x", [BPC, 128, L], mm_dt, kind="ExternalInput")
    w_d = nc.dram_tensor("w", [128, KT * 128], w_dt, kind="ExternalInput")
    bias_d = nc.dram_tensor("bias", [128, 1], mybir.dt.float32, kind="ExternalInput")
    out_d = nc.dram_tensor("out", [BPC, 128, LOUT], out_dt, kind="ExternalOutput")
    f8 = mybir.dt.float8e4
    if fp8taps:
        # xq is padded to L+1 cols so the shifted plane-1 load stays in
        # bounds; wq holds the 2-tap stationary blocks [p, (half ktile) f].
        xq_d = nc.dram_tensor("xq", [BPC, 128, L + 1], f8, kind="ExternalInput")
        wq_d = nc.dram_tensor("wq", [128, 256], f8, kind="ExternalInput")

    f32 = mybir.dt.float32
    ident = mybir.ActivationFunctionType.Identity
    oeng = getattr(nc, out_eng)
    beng = getattr(nc, bias_eng)
    xe = getattr(nc, xeng)

    with TileContext(nc) as tc:
        with (
            tc.tile_pool(name="wpool", bufs=1) as wpool,
            tc.tile_pool(name="xpool", bufs=xbufs) as xpool,
            tc.tile_pool(name="opool", bufs=obufs) as opool,
            tc.tile_pool(name="pspool", bufs=psbufs, space="PSUM") as pspool,
        ):
            # SP queue carries wt first (warmup fodder), then all x loads.
            wt = wpool.tile([128, KT * 128], w_dt)
            if wsplit:
                # First tap's weight block lands first so matmul 0 can start
                # as soon as X chunk 0 is in; the rest follows on the queue.
                nc.sync.dma_start(wt[:, 0:128], w_d[:, 0:128])
                nc.sync.dma_start(wt[:, 128:], w_d[:, 128:])
            else:
                nc.sync.dma_start(wt[:], w_d[:])
            if fp8taps:
                # fp8 operands ride the ACT HWDGE queue: it is idle during
                # the load phase (out-DMAs only start after the first evac),
                # so the prologue stays as short as the bf16 baseline's.
                wq_t = wpool.tile([128, 4, 64], f8)
                nc.scalar.dma_start(
                    wq_t[:], wq_d[:].rearrange("p (four f) -> p four f", four=4)
                )
            bias_t = wpool.tile([128, 1], f32)
            beng.dma_start(bias_t[:], bias_d[:])

            if warmup:
                # Ramp the PE p-state while the first X/W DMAs are in
                # flight: matmuls on a zeroed SBUF tile (no DMA dependency,
                # can start ~immediately) into a dedicated PSUM bank.
                wu = wpool.tile([128, 512], mm_dt)
                nc.vector.memset(wu[:], 0)
                if korder:
                    wps = pspool.tile([128, 512], f32, tag="ps")
                else:
                    wps = pspool.tile([128, 512], f32, tag="wps", bufs=1)
                for _ in range(warmup):
                    nc.tensor.matmul(
                        wps[:, :warmup_rows], wu[:, 0:128], wu[:, 0:warmup_rows],
                        start=True, stop=True, skip_group_check=True,
                    )

            import contextlib

            loop_cm = (
                tc.For_i(0, loop_repeat, 1)
                if loop_repeat is not None
                else contextlib.nullcontext()
            )
            n_evac = 0
            with loop_cm:
              for _rep in range(repeat):
                for b in range(BPC):
                    # Whole-batch X in SBUF; DMA'd in chunks so the first
                    # tile's matmuls only wait on chunk 0.
                    xt = xpool.tile([128, L], mm_dt, tag="xt")
                    xqt = None
                    if fp8taps:
                        xqt = xpool.tile([128, 2, L], f8, tag="xqt")
                    cuts = [0, first_chunk] if (b == 0 and first_chunk) else [0]
                    rest = (L - cuts[-1]) // max(1, xchunks - len(cuts) + 1)
                    while cuts[-1] + rest < L:
                        cuts.append(cuts[-1] + rest)
                    cuts.append(L)
                    for ci, (c0, c1) in enumerate(zip(cuts[:-1], cuts[1:])):
                        eng = xe
                        if x0eng is not None and b == 0 and ci == 0:
                            # First chunk rides the (idle at startup) out-DMA
                            # queue so it doesn't wait behind the weight load.
                            eng = getattr(nc, x0eng)
                        eng.dma_start(xt[:, c0:c1], x_d[b, :, c0:c1])
                    if fp8taps:
                        # Two shifted planes (cols j and j+1) so the
                        # DoubleRow k-tile dim is a plain non-overlapping
                        # tile dim. Few big chunks; chunk 0 small on b==0
                        # so the first DR isn't stalled. SP queue: ACT is
                        # loaded with out-DMAs + the psb bias staging op.
                        qcuts = (
                            [0, 520, 2048, L] if b == 0 else [0, L // 2, L]
                        )
                        for c0, c1 in zip(qcuts[:-1], qcuts[1:]):
                            for pl in (0, 1):
                                nc.sync.dma_start(
                                    xqt[:, pl, c0:c1],
                                    xq_d[b, :, c0 + pl : c1 + pl],
                                )
                    # Tile bounds; optionally thin final tile to cut the tail.
                    bounds = [(j * TL, min(TL, LOUT - j * TL)) for j in range(NLT)]
                    if tail and b == BPC - 1:
                        l0, t = bounds[-1]
                        bounds[-1] = (l0, t - tail)
                        bounds.append((l0 + t - tail, tail))
                    def _evac(ps, l0, t, n):
                        ot = opool.tile([128, TL], out_dt, tag="ot")
                        use_dve = (
                            evac == "dve" or (evac == "alt" and n % 2 == 1)
                        )
                        if use_dve:
                            nc.vector.tensor_scalar_add(
                                ot[:, :t], ps[:, :t], bias_t[:]
                            )
                        else:
                            nc.scalar.activation(
                                ot[:, :t], ps[:, :t], ident, bias=bias_t[:]
                            )
                        oeng.dma_start(out_d[b, :, l0 : l0 + t], ot[:, :t])

                    if korder:
                        # tap-outer over groups of `korder` tiles: one
                        # weight load serves the whole group (4x fewer
                        # LdWeights -> less SBUF read pressure).
                        for g0 in range(0, len(bounds), korder):
                            grp = bounds[g0 : g0 + korder]
                            pss = []
                            for _ in grp:
                                ps_j = pspool.tile([128, TL], f32, tag="ps")
                                pss.append(ps_j)
                            for k in range(KT):
                                for ps_j, (l0, t) in zip(pss, grp):
                                    nc.tensor.matmul(
                                        ps_j[:, :t],
                                        wt[:, k * 128 : (k + 1) * 128],
                                        xt[:, l0 + k : l0 + k + t],
                                        start=(k == 0),
                                        stop=(k == KT - 1),
                                        skip_group_check=True,
                                    )
                            for ps_j, (l0, t) in zip(pss, grp):
                                _evac(ps_j, l0, t, n_evac)
                                n_evac += 1
                    elif fp8taps:
                      # Taps {0,1} as fp8 DoubleRow (contraction 256, 2x PE
                      # rate): real half accumulates into the main bank's
                      # partitions 0:64 (ISA: DR dst must start at 0), imag
                      # half into a second bank, folded in at evacuation.
                      for l0, t in bounds:
                        ps = pspool.tile([128, TL], f32, tag="ps")
                        psb = pspool.tile([128, TL], f32, tag="psb", bufs=2)
                        xdr = xqt[:, :, l0 : l0 + t]
                        # bf16 taps first: their operands (SP queue) are
                        # ready before the fp8 ones (ACT queue) at startup.
                        for k in range(2, KT):
                            nc.tensor.matmul(
                                ps[:, :t],
                                wt[:, k * 128 : (k + 1) * 128],
                                xt[:, l0 + k : l0 + k + t],
                                start=(k == 2),
                                stop=False,
                                skip_group_check=True,
                            )
                        nc.tensor.matmul(
                            psb[0:64, :t], wq_t[:, 2:4, :], xdr,
                            start=True, stop=True,
                            perf_mode=mybir.MatmulPerfMode.DoubleRow,
                            skip_group_check=True,
                        )
                        nc.tensor.matmul(
                            ps[0:64, :t], wq_t[:, 0:2, :], xdr,
                            start=False, stop=True,
                            perf_mode=mybir.MatmulPerfMode.DoubleRow,
                            skip_group_check=True,
                        )
                        # evac: real = psA+bias_r (DVE). imag: ACT stages
                        # tmp = psB+bias_i (one PSUM input), DVE merges
                        # tmp + psA_hi (one PSUM input) — the HW verifier
                        # allows only one PSUM operand per DVE/ACT op.
                        ot = opool.tile([128, TL], out_dt, tag="ot")
                        tmp = opool.tile([64, TL], f32, tag="tmp", bufs=3)
                        nc.scalar.activation(
                            tmp[:, :t], psb[0:64, :t], ident,
                            bias=bias_t[64:128],
                        )
                        nc.vector.tensor_scalar_add(
                            ot[0:64, :t], ps[0:64, :t], bias_t[0:64]
                        )
                        nc.vector.scalar_tensor_tensor(
                            ot[64:128, :t],
                            tmp[:, :t],
                            0.0,
                            ps[64:128, :t],
                            mybir.AluOpType.add,
                            mybir.AluOpType.add,
                        )
                        oeng.dma_start(out_d[b, :, l0 : l0 + t], ot[:, :t])
                        n_evac += 1
                    else:
                      for l0, t in bounds:
                        ps = pspool.tile([128, TL], f32, tag="ps")
                        for k in range(KT):
                            nc.tensor.matmul(
                                ps[:, :t],
                                wt[:, k * 128 : (k + 1) * 128],
                                xt[:, l0 + k : l0 + k + t],
                                start=(k == 0),
                                stop=(k == KT - 1),
                            )
                        _evac(ps, l0, t, n_evac)
                        n_evac += 1

    nc.compile()
    return nc


def _pack(x_real, x_imag, kernel_real, kernel_imag, bias_real, bias_imag, np_dt,
          w_np_dt=None, fp8taps=0):
    if w_np_dt is None:
        w_np_dt = np_dt
    X = np.empty((B, 128, L), np_dt)
    X[:, :CIN] = x_real.transpose(0, 2, 1)
    X[:, CIN:] = x_imag.transpose(0, 2, 1)
    Wk = np.empty((KT, 128, 128), np.float32)
    Wk[:, :CIN, :F] = kernel_real
    Wk[:, :CIN, F:] = kernel_imag
    Wk[:, CIN:, :F] = -kernel_imag
    Wk[:, CIN:, F:] = kernel_real
    bias2 = (
        np.concatenate([bias_real, bias_imag]).reshape(128, 1).astype(np.float32)
    )
    extra = {}
    if fp8taps:
        f8np = mybir.dt.np(mybir.dt.float8e4)
        Xf = np.zeros((B, 128, L + 1), np.float32)
        Xf[:, :CIN, :L] = x_real.transpose(0, 2, 1)
        Xf[:, CIN:, :L] = x_imag.transpose(0, 2, 1)
        extra["xq"] = np.ascontiguousarray((Xf * 2.0**XSH).astype(f8np))
        # wq layout [128, 2*128]: half h (real/imag out block), ktile-major:
        # wq[p, h*128 + i*64 + f] = Wk[i, p, h*64 + f] * 2^WSH
        Wq = np.empty((128, 256), np.float32)
        for h in (0, 1):
            for i in (0, 1):
                Wq[:, h * 128 + i * 64 : h * 128 + (i + 1) * 64] = (
                    Wk[i, :, h * 64 : (h + 1) * 64]
                )
        extra["wq"] = np.ascontiguousarray((Wq * 2.0**WSH).astype(f8np))
        # bf16 taps carry the product scale; taps 0,1 unused -> zero
        Wk = Wk * 2.0**PSH
        Wk[0] = 0.0
        Wk[1] = 0.0
        bias2 = bias2 * 2.0**PSH
    W2 = Wk.transpose(1, 0, 2).reshape(128, KT * 128).astype(w_np_dt)
    return X, np.ascontiguousarray(W2), bias2, extra


def _parse_dt(name):
    name = name or MM_DT_NAME
    if "," in name:
        xn, wn = name.split(",")
    else:
        xn = wn = name
    return getattr(mybir.dt, xn), getattr(mybir.dt, wn)


def _prepare(inputs, mm_dt_name=None, out_dt_name=None, build_kw=None):
    mm_dt, w_dt = _parse_dt(mm_dt_name)
    out_dt = getattr(mybir.dt, out_dt_name or OUT_DT_NAME)
    np_dt = mybir.dt.np(mm_dt)
    w_np_dt = mybir.dt.np(w_dt)
    build_kw = {**DEFAULT_BUILD_KW, **(build_kw or {})}
    fp8taps = build_kw.get("fp8taps", 0)
    args = {
        k: np.asarray(inputs[k], np.float32)
        for k in (
            "x_real", "x_imag", "kernel_real", "kernel_imag", "bias_real", "bias_imag",
        )
    }
    X, W2, bias2, extra = _pack(
        np_dt=np_dt, w_np_dt=w_np_dt, fp8taps=fp8taps, **args
    )

    nc = _build_nc(mm_dt, w_dt=w_dt, out_dt=out_dt, **(build_kw or {}))
    in_maps = [
        {
            "x": np.ascontiguousarray(X[i * BPC : (i + 1) * BPC]),
            "w": W2,
            "bias": bias2,
            **(
                {
                    "xq": np.ascontiguousarray(
                        extra["xq"][i * BPC : (i + 1) * BPC]
                    ),
                    "wq": extra["wq"],
                }
                if fp8taps
                else {}
            ),
        }
        for i in range(NCORES)
    ]
    return nc, in_maps


def _gather(results, postscale=1.0):
    O = np.concatenate([np.asarray(r["out"], np.float32) for r in results], axis=0)
    if postscale != 1.0:
        O = O * postscale
    O = O.reshape(B, 2, F, LOUT).transpose(0, 3, 2, 1)  # [B, LOUT, F, 2]
    return np.ascontiguousarray(O, dtype=np.float32)


def _run(inputs, trace=False, mm_dt_name=None, out_dt_name=None, build_kw=None):
    build_kw = {**DEFAULT_BUILD_KW, **(build_kw or {})}
    nc, in_maps = _prepare(inputs, mm_dt_name, out_dt_name, build_kw)
    res = run_bass_kernel_spmd(nc, in_maps, core_ids=list(range(NCORES)), trace=trace)
    postscale = 2.0 ** -PSH if build_kw.get("fp8taps", 0) else 1.0
    return _gather(res.results, postscale), res


def kernel(**inputs) -> np.ndarray:
    out, _ = _run(inputs, trace=False)
    return out



# revision 36
# speedup vs baseline: 1.0116x; 1.0116x over previous
"""Complex Conv1D (VALID, stride 1) on Trainium2 — Bass/Tile, 8-core data-parallel.

Problem (hardcoded shapes):
  x_real/x_imag: [32, 4096, 64] f32, kernel_real/imag: [9, 64, 64] f32,
  bias_real/imag: [64] f32  ->  out [32, 4088, 64, 2] f32
  out_real = conv(xr, wr) - conv(xi, wi) + br
  out_imag = conv(xr, wi) + conv(xi, wr) + bi

Mapping: complex multiply as its 2x2 real block-matrix form so each tap is ONE
full 128-contract matmul:
  X_b [128, L]   rows 0:64 = xr[b].T (channels on partitions), 64:128 = xi[b].T
  W[k] [128,128] = [[wr[k], wi[k]], [-wi[k], wr[k]]]
  psum[128, T] += W[k].T @ X_b[:, l0+k : l0+k+T]   for k = 0..8
  psum rows 0:64 = real output (filters), rows 64:128 = imag output.
Batch is sharded 4-per-core across 8 cores; weights replicated. The kernel
emits the output transposed as [b, 128, L_out]; the host restores
[B, L_out, F, 2].

PE does 9 rows (128x128 MACs each) per output position — 9*4088*4 = 147k rows
per core = 61.4us at 2.4GHz, the hard floor. The rest of the design keeps the
PE near that floor (HW-measured choices, each A/B'd via a repeat-loop diff):
  - bf16 operands (same 1 cycle/row as f32r, half the SBUF/DMA traffic;
    rel err ~2.3e-3 vs the f32 reference, gate is 2e-2). f32 outputs.
  - whole-batch X tiles DMA'd in 8 chunks on the SP queue: spreading the
    transfers reduced measured DMA<->PE SBUF contention vs one big burst.
  - evacuation psum->SBUF on the DVE (vector) engine, not Act: measured
    ~5us less PE interference; out-DMAs ride the Act queue; bias load on
    the gpsimd/SWDGE path to keep the startup HWDGE queue clear.
  - warmup matmuls measured net-negative (they delay the real stream
    more than the p-state ramp costs), so warmup defaults to 0.
  - prologue: W DMA split so tap 0's block lands first, X chunk 0 on the
    (startup-idle) ACT queue in parallel with W on SP; 8 PSUM banks; the
    final tile thinned (tail=256) to shorten the end-of-kernel drain
    (last evac + out-DMA + completion + exit barrier, ~4us in CoreSim).

Explored and rejected (2026-08-10 session): a partial-fp8 path (fp8taps=2)
that runs taps {0,1} as one fp8-e4m3 DoubleRow matmul pair — contraction
256, dst partitions 0:64 (ISA requires base 0), imag half in a second PSUM
bank folded in at evacuation via an ACT staging op (HW verifier allows only
one PSUM operand per DVE/ACT op). It is numerically fine (rel err 1.78e-2,
gate 2e-2) and ~9% faster in the CoreSim cost model (which prices DoubleRow
at 0.5 cycles/row), but measured +3us/iter SLOWER on hardware, and a
64-matmul microbench shows DR streams columns ~1.4x slower than bf16 here —
DoubleRow does not double per-column throughput on this silicon/runtime.
The path is kept behind build_kw={'fp8taps': 2}.

Timing methodology (axon tunnel, no NTFF profiling): per-call wall times
carry a large drifting dispatch overhead (3-90ms!), so per-iteration HW
time is estimated by pairing single calls of loop_repeat=1 and =401 builds
(hardware For_i around the whole kernel) and taking the median of paired
(tR-t1)/400 estimates — see timing.py. Machine-state drift of +-4% between
sessions is normal; A/B deltas under ~3us/iter are not resolvable.
"""

import numpy as np

import concourse.bacc as bacc
import concourse.bass as bass
import concourse.mybir as mybir
from concourse.tile import TileContext
from concourse.bass_utils import run_bass_kernel_spmd

B, L, CIN, KT, F = 32, 4096, 64, 9, 64
LOUT = L - KT + 1  # 4088
NCORES = 8
BPC = B // NCORES  # batches per core
TL = 512  # output-tile width (one PSUM bank of fp32)
NLT = (LOUT + TL - 1) // TL  # 8

MM_DT_NAME = "bfloat16"
OUT_DT_NAME = "float32"

# Partial-fp8 scheme (fp8taps=2): taps {0,1} run as one fp8-e4m3 DoubleRow
# matmul pair (contraction 256, 2x PE rate), taps 2..8 stay bf16. Scales keep
# one PSUM accumulation consistent: fp8 X is pre-scaled by 2^XSH and fp8 W by
# 2^WSH (product 2^PSH), bf16-tap weights and the bias by 2^PSH, and the host
# multiplies the output by 2^-PSH. Measured rel err 1.78e-2 (gate 2e-2).
XSH, WSH = 4, 6
PSH = XSH + WSH

# Default build config. bf16 taps only: the partial-fp8 DoubleRow path
# (fp8taps=2) is correct on HW (rel err 1.78e-2) and ~5.6us faster in the
# cost-model sim, but measured +3us SLOWER on hardware — DoubleRow does not
# deliver 2x per-column throughput on this silicon/runtime, so the extra xq
# DMAs and the two-bank evacuation are pure overhead. Kept as an option.
# The micro-opts here are sim-positive and HW-noise-neutral: thinned final
# tile (shorter end-of-kernel DMA drain), split W prologue DMA + first X
# chunk on the idle ACT queue (parallel prologue), all 8 PSUM banks.
DEFAULT_BUILD_KW = {
    "fp8taps": 0,
    "tail": 256,
    "wsplit": 1,
    "x0eng": "scalar",
    "psbufs": 8,
}


def _build_nc(
    mm_dt,
    w_dt=None,
    out_dt=None,
    xbufs=3,
    obufs=4,
    psbufs=6,
    warmup=0,
    warmup_rows=512,
    xchunks=8,
    first_chunk=512,
    xeng="sync",
    out_eng="scalar",
    bias_eng="gpsimd",
    tail=0,
    korder=0,
    evac="dve",
    repeat=1,
    loop_repeat=None,
    wsplit=0,
    x0eng=None,
    fp8taps=0,
):
    nc = bacc.Bacc("TRN2", target_bir_lowering=False, debug=False, num_devices=NCORES)
    if w_dt is None:
        w_dt = mm_dt
    if out_dt is None:
        out_dt = getattr(mybir.dt, OUT_DT_NAME)

    x_d = nc.dram_tensor("x", [BPC, 128, L], mm_dt, kind="ExternalInput")
    w_d = nc.dram_tensor("w", [128, KT * 128], w_dt, kind="ExternalInput")
    bias_d = nc.dram_tensor("bias", [128, 1], mybir.dt.float32, kind="ExternalInput")
    out_d = nc.dram_tensor("out", [BPC, 128, LOUT], out_dt, kind="ExternalOutput")
    f8 = mybir.dt.float8e4
    if fp8taps:
        # xq is padded to L+1 cols so the shifted plane-1 load stays in
        # bounds; wq holds the 2-tap stationary blocks [p, (half ktile) f].
        xq_d = nc.dram_tensor("xq", [BPC, 128, L + 1], f8, kind="ExternalInput")
        wq_d = nc.dram_tensor("wq", [128, 256], f8, kind="ExternalInput")

    f32 = mybir.dt.float32
    ident = mybir.ActivationFunctionType.Identity
    oeng = getattr(nc, out_eng)
    beng = getattr(nc, bias_eng)
    xe = getattr(nc, xeng)

    with TileContext(nc) as tc:
        with (
            tc.tile_pool(name="wpool", bufs=1) as wpool,
            tc.tile_pool(name="xpool", bufs=xbufs) as xpool,
            tc.tile_pool(name="opool", bufs=obufs) as opool,
            tc.tile_pool(name="pspool", bufs=psbufs, space="PSUM") as pspool,
        ):
            # SP queue carries wt first (warmup fodder), then all x loads.
            wt = wpool.tile([128, KT * 128], w_dt)
            if wsplit:
                # First tap's weight block lands first so matmul 0 can start
                # as soon as X chunk 0 is in; the rest follows on the queue.
                nc.sync.dma_start(wt[:, 0:128], w_d[:, 0:128])
                nc.sync.dma_start(wt[:, 128:], w_d[:, 128:])
            else:
                nc.sync.dma_start(wt[:], w_d[:])
            if fp8taps:
                # fp8 operands ride the ACT HWDGE queue: it is idle during
                # the load phase (out-DMAs only start after the first evac),
                # so the prologue stays as short as the bf16 baseline's.
                wq_t = wpool.tile([128, 4, 64], f8)
                nc.scalar.dma_start(
                    wq_t[:], wq_d[:].rearrange("p (four f) -> p four f", four=4)
                )
            bias_t = wpool.tile([128, 1], f32)
            beng.dma_start(bias_t[:], bias_d[:])

            if warmup:
                # Ramp the PE p-state while the first X/W DMAs are in
                # flight: matmuls on a zeroed SBUF tile (no DMA dependency,
                # can start ~immediately) into a dedicated PSUM bank.
                wu = wpool.tile([128, 512], mm_dt)
                nc.vector.memset(wu[:], 0)
                if korder:
                    wps = pspool.tile([128, 512], f32, tag="ps")
                else:
                    wps = pspool.tile([128, 512], f32, tag="wps", bufs=1)
                for _ in range(warmup):
                    nc.tensor.matmul(
                        wps[:, :warmup_rows], wu[:, 0:128], wu[:, 0:warmup_rows],
                        start=True, stop=True, skip_group_check=True,
                    )

            import contextlib

            loop_cm = (
                tc.For_i(0, loop_repeat, 1)
                if loop_repeat is not None
                else contextlib.nullcontext()
            )
            n_evac = 0
            with loop_cm:
              for _rep in range(repeat):
                for b in range(BPC):
                    # Whole-batch X in SBUF; DMA'd in chunks so the first
                    # tile's matmuls only wait on chunk 0.
                    xt = xpool.tile([128, L], mm_dt, tag="xt")
                    xqt = None
                    if fp8taps:
                        xqt = xpool.tile([128, 2, L], f8, tag="xqt")
                    cuts = [0, first_chunk] if (b == 0 and first_chunk) else [0]
                    rest = (L - cuts[-1]) // max(1, xchunks - len(cuts) + 1)
                    while cuts[-1] + rest < L:
                        cuts.append(cuts[-1] + rest)
                    cuts.append(L)
                    for ci, (c0, c1) in enumerate(zip(cuts[:-1], cuts[1:])):
                        eng = xe
                        if x0eng is not None and b == 0 and ci == 0:
                            # First chunk rides the (idle at startup) out-DMA
                            # queue so it doesn't wait behind the weight load.
                            eng = getattr(nc, x0eng)
                        eng.dma_start(xt[:, c0:c1], x_d[b, :, c0:c1])
                    if fp8taps:
                        # Two shifted planes (cols j and j+1) so the
                        # DoubleRow k-tile dim is a plain non-overlapping
                        # tile dim. Few big chunks; chunk 0 small on b==0
                        # so the first DR isn't stalled. SP queue: ACT is
                        # loaded with out-DMAs + the psb bias staging op.
                        qcuts = (
                            [0, 520, 2048, L] if b == 0 else [0, L // 2, L]
                        )
                        for c0, c1 in zip(qcuts[:-1], qcuts[1:]):
                            for pl in (0, 1):
                                nc.sync.dma_start(
                                    xqt[:, pl, c0:c1],
                                    xq_d[b, :, c0 + pl : c1 + pl],
                                )
                    # Tile bounds; optionally thin final tile to cut the tail.
                    bounds = [(j * TL, min(TL, LOUT - j * TL)) for j in range(NLT)]
                    if tail and b == BPC - 1:
                        l0, t = bounds[-1]
                        bounds[-1] = (l0, t - tail)
                        bounds.append((l0 + t - tail, tail))
                    def _evac(ps, l0, t, n):
                        ot = opool.tile([128, TL], out_dt, tag="ot")
                        use_dve = (
                            evac == "dve" or (evac == "alt" and n % 2 == 1)
                        )
                        if use_dve:
                            nc.vector.tensor_scalar_add(
                                ot[:, :t], ps[:, :t], bias_t[:]
                            )
                        else:
                            nc.scalar.activation(
                                ot[:, :t], ps[:, :t], ident, bias=bias_t[:]
                            )
                        oeng.dma_start(out_d[b, :, l0 : l0 + t], ot[:, :t])

                    if korder:
                        # tap-outer over groups of `korder` tiles: one
                        # weight load serves the whole group (4x fewer
                        # LdWeights -> less SBUF read pressure).
                        for g0 in range(0, len(bounds), korder):
                            grp = bounds[g0 : g0 + korder]
                            pss = []
                            for _ in grp:
                                ps_j = pspool.tile([128, TL], f32, tag="ps")
                                pss.append(ps_j)
                            for k in range(KT):
                                for ps_j, (l0, t) in zip(pss, grp):
                                    nc.tensor.matmul(
                                        ps_j[:, :t],
                                        wt[:, k * 128 : (k + 1) * 128],
                                        xt[:, l0 + k : l0 + k + t],
                                        start=(k == 0),
                                        stop=(k == KT - 1),
                                        skip_group_check=True,
                                    )
                            for ps_j, (l0, t) in zip(pss, grp):
                                _evac(ps_j, l0, t, n_evac)
                                n_evac += 1
                    elif fp8taps:
                      # Taps {0,1} as fp8 DoubleRow (contraction 256, 2x PE
                      # rate): real half accumulates into the main bank's
                      # partitions 0:64 (ISA: DR dst must start at 0), imag
                      # half into a second bank, folded in at evacuation.
                      for l0, t in bounds:
                        ps = pspool.tile([128, TL], f32, tag="ps")
                        psb = pspool.tile([128, TL], f32, tag="psb", bufs=2)
                        xdr = xqt[:, :, l0 : l0 + t]
                        # bf16 taps first: their operands (SP queue) are
                        # ready before the fp8 ones (ACT queue) at startup.
                        for k in range(2, KT):
                            nc.tensor.matmul(
                                ps[:, :t],
                                wt[:, k * 128 : (k + 1) * 128],
                                xt[:, l0 + k : l0 + k + t],
                                start=(k == 2),
                                stop=False,
                                skip_group_check=True,
                            )
                        nc.tensor.matmul(
                            psb[0:64, :t], wq_t[:, 2:4, :], xdr,
                            start=True, stop=True,
                            perf_mode=mybir.MatmulPerfMode.DoubleRow,
                            skip_group_check=True,
                        )
                        nc.tensor.matmul(
                            ps[0:64, :t], wq_t[:, 0:2, :], xdr,
                            start=False, stop=True,
                            perf_mode=mybir.MatmulPerfMode.DoubleRow,
                            skip_group_check=True,
                        )
                        # evac: real = psA+bias_r (DVE). imag: ACT stages
                        # tmp = psB+bias_i (one PSUM input), DVE merges
                        # tmp + psA_hi (one PSUM input) — the HW verifier
                        # allows only one PSUM operand per DVE/ACT op.
                        ot = opool.tile([128, TL], out_dt, tag="ot")
                        tmp = opool.tile([64, TL], f32, tag="tmp", bufs=3)
                        nc.scalar.activation(
                            tmp[:, :t], psb[0:64, :t], ident,
                            bias=bias_t[64:128],
                        )
                        nc.vector.tensor_scalar_add(
                            ot[0:64, :t], ps[0:64, :t], bias_t[0:64]
                        )
                        nc.vector.scalar_tensor_tensor(
                            ot[64:128, :t],
                            tmp[:, :t],
                            0.0,
                            ps[64:128, :t],
                            mybir.AluOpType.add,
                            mybir.AluOpType.add,
                        )
                        oeng.dma_start(out_d[b, :, l0 : l0 + t], ot[:, :t])
                        n_evac += 1
                    else:
                      for l0, t in bounds:
                        ps = pspool.tile([128, TL], f32, tag="ps")
                        for k in range(KT):
                            nc.tensor.matmul(
                                ps[:, :t],
                                wt[:, k * 128 : (k + 1) * 128],
                                xt[:, l0 + k : l0 + k + t],
                                start=(k == 0),
                                stop=(k == KT - 1),
                            )
                        _evac(ps, l0, t, n_evac)
                        n_evac += 1

    nc.compile()
    return nc


def _pack(x_real, x_imag, kernel_real, kernel_imag, bias_real, bias_imag, np_dt,
          w_np_dt=None, fp8taps=0):
    if w_np_dt is None:
        w_np_dt = np_dt
    X = np.empty((B, 128, L), np_dt)
    X[:, :CIN] = x_real.transpose(0, 2, 1)
    X[:, CIN:] = x_imag.transpose(0, 2, 1)
    Wk = np.empty((KT, 128, 128), np.float32)
    Wk[:, :CIN, :F] = kernel_real
    Wk[:, :CIN, F:] = kernel_imag
    Wk[:, CIN:, :F] = -kernel_imag
    Wk[:, CIN:, F:] = kernel_real
    bias2 = (
        np.concatenate([bias_real, bias_imag]).reshape(128, 1).astype(np.float32)
    )
    extra = {}
    if fp8taps:
        f8np = mybir.dt.np(mybir.dt.float8e4)
        Xf = np.zeros((B, 128, L + 1), np.float32)
        Xf[:, :CIN, :L] = x_real.transpose(0, 2, 1)
        Xf[:, CIN:, :L] = x_imag.transpose(0, 2, 1)
        extra["xq"] = np.ascontiguousarray((Xf * 2.0**XSH).astype(f8np))
        # wq layout [128, 2*128]: half h (real/imag out block), ktile-major:
        # wq[p, h*128 + i*64 + f] = Wk[i, p, h*64 + f] * 2^WSH
        Wq = np.empty((128, 256), np.float32)
        for h in (0, 1):
            for i in (0, 1):
                Wq[:, h * 128 + i * 64 : h * 128 + (i + 1) * 64] = (
                    Wk[i, :, h * 64 : (h + 1) * 64]
                )
        extra["wq"] = np.ascontiguousarray((Wq * 2.0**WSH).astype(f8np))
        # bf16 taps carry the product scale; taps 0,1 unused -> zero
        Wk = Wk * 2.0**PSH
        Wk[0] = 0.0
        Wk[1] = 0.0
        bias2 = bias2 * 2.0**PSH
    W2 = Wk.transpose(1, 0, 2).reshape(128, KT * 128).astype(w_np_dt)
    return X, np.ascontiguousarray(W2), bias2, extra


def _parse_dt(name):
    name = name or MM_DT_NAME
    if "," in name:
        xn, wn = name.split(",")
    else:
        xn = wn = name
    return getattr(mybir.dt, xn), getattr(mybir.dt, wn)


def _prepare(inputs, mm_dt_name=None, out_dt_name=None, build_kw=None):
    mm_dt, w_dt = _parse_dt(mm_dt_name)
    out_dt = getattr(mybir.dt, out_dt_name or OUT_DT_NAME)
    np_dt = mybir.dt.np(mm_dt)
    w_np_dt = mybir.dt.np(w_dt)
    build_kw = {**DEFAULT_BUILD_KW, **(build_kw or {})}
    fp8taps = build_kw.get("fp8taps", 0)
    args = {
        k: np.asarray(inputs[k], np.float32)
        for k in (
            "x_real", "x_imag", "kernel_real", "kernel_imag", "bias_real", "bias_imag",
        )
    }
    X, W2, bias2, extra = _pack(
        np_dt=np_dt, w_np_dt=w_np_dt, fp8taps=fp8taps, **args
    )

    nc = _build_nc(mm_dt, w_dt=w_dt, out_dt=out_dt, **(build_kw or {}))
    in_maps = [
        {
            "x": np.ascontiguousarray(X[i * BPC : (i + 1) * BPC]),
            "w": W2,
            "bias": bias2,
            **(
                {
                    "xq": np.ascontiguousarray(
                        extra["xq"][i * BPC : (i + 1) * BPC]
                    ),
                    "wq": extra["wq"],
                }
                if fp8taps
                else {}
            ),
        }
        for i in range(NCORES)
    ]
    return nc, in_maps


def _gather(results, postscale=1.0):
    O = np.concatenate([np.asarray(r["out"], np.float32) for r in results], axis=0)
    if postscale != 1.0:
        O = O * postscale
    O = O.reshape(B, 2, F, LOUT).transpose(0, 3, 2, 1)  # [B, LOUT, F, 2]
    return np.ascontiguousarray(O, dtype=np.float32)


def _run(inputs, trace=False, mm_dt_name=None, out_dt_name=None, build_kw=None):
    build_kw = {**DEFAULT_BUILD_KW, **(build_kw or {})}
    nc, in_maps = _prepare(inputs, mm_dt_name, out_dt_name, build_kw)
    res = run_bass_kernel_spmd(nc, in_maps, core_ids=list(range(NCORES)), trace=trace)
    postscale = 2.0 ** -PSH if build_kw.get("fp8taps", 0) else 1.0
    return _gather(res.results, postscale), res


def kernel(**inputs) -> np.ndarray:
    out, _ = _run(inputs, trace=False)
    return out

